# revision 21
# baseline (speedup 1.0000x reference)
"""Patch TileContext._drain_and_barrier: this container's walrus codegen
rejects >2 sem waits on one CTRL (Drain) instruction. Split the kernel-tail
drain's waits across separate nop instructions (1 wait each)."""
import concourse.tile as tile  # noqa
import concourse.mybir as mybir
from concourse.vector_clock import ScopedClock
from concourse._compat import not_none as nn


def _drain_and_barrier_split(self, tick_clock, wait_clock):
    nc = self.nc
    carrier = nc.sync.nop()
    wait_clock.add_sem_waits(carrier.ins, ScopedClock({None: tick_clock.global_clock}))
    si = carrier.ins.sync_info
    waits = list(si.on_wait) if si and si.on_wait else []
    if len(waits) > 1:
        si.on_wait.clear()
        si.on_wait.append(waits[0])
        for w in waits[1:]:
            n2 = nc.sync.nop()
            n2.ins.sync_info = mybir.SyncInfo(on_wait=[w], on_update=[])
    nc.sync.drain()

    nc.all_engine_barrier()
    assert self.sems is not None
    popped = nc._tile_sem_poison_stack.pop()
    assert popped is self._sem_poison
    nc.clear_and_free_semaphores(list(self.sems.allocated().values()))
    nc.all_engine_barrier()


tile.TileContext._drain_and_barrier = _drain_and_barrier_split


# ---- global wait-splitting pass ----
# This walrus build packs at most MAX_WAITS sem-waits per instruction
# (ISA EVENTS struct holds one; codegen can prepend a limited number of
# sync-wait commands). Move excess waits onto InstNoOp carriers.
MAX_WAITS = 2

def fix_waits(nc, max_waits=MAX_WAITS):
    import concourse.mybir as mybir
    dma2 = getattr(nc, "_fix_dma_waits2", False)
    n_fixed = 0
    for fn in nc.m.functions:
        for blk in fn.blocks:
            insts = blk.instructions
            out = []
            for inst in insts:
                lim = max_waits
                if dma2 and isinstance(inst, mybir.InstDMACopy):
                    lim = 2
                si = getattr(inst, "sync_info", None)
                if si is not None and si.on_wait and len(si.on_wait) > lim:
                    waits = list(si.on_wait)
                    si.on_wait.clear()
                    for w in waits[:-lim] if lim else waits:
                        n_fixed += 1
                        nop = mybir.InstNoOp(
                            name=f"{inst.name}.wsplit{n_fixed}",
                            sync_info=mybir.SyncInfo(on_wait=[w], on_update=[]),
                            bass_nofuse=True,
                            engine=inst.engine,
                        )
                        out.append(nop)
                    for w in waits[-lim:] if lim else []:
                        si.on_wait.append(w)
                out.append(inst)
            blk.instructions = out
    return n_fixed


# auto-apply fix_waits on serialization
import concourse.bass as _bass
_orig_to_json_bytes = _bass.Bass.to_json_bytes

def _to_json_bytes_fixed(self, *a, **kw):
    try:
        fix_waits(self, max_waits=getattr(self, "_fix_max_waits", 1))
    except Exception:
        import traceback; traceback.print_exc()
    return _orig_to_json_bytes(self, *a, **kw)

_bass.Bass.to_json_bytes = _to_json_bytes_fixed


"""NodeModel GNN kernel for Trainium2 (Bass/Tile), 8-core SPMD — v2.

Design (vs v1 baseline):
- Destination nodes sharded 8 ways (contiguous ranges, no collectives).
- Host pre-gathers & pre-transposes the per-edge source features into a
  bf16 stream [128, S]: rows 0-63 = x[src]^T per slot, rows 64-127 =
  x[dst]^T per slot. Sequential DMA at full bandwidth replaces the
  descriptor-rate-limited indirect gathers of v1.
- All matmuls bf16 (4x PE throughput vs fp32): h1 = relu(W1^T @ [xs;xc])
  is ONE K=128 matmul per round since lhsT = W1 as-is.
- Columns = destination nodes grouped by exact degree (shared histogram-max
  schedule across cores for SPMD); rounds process the r-th edge of each
  still-active column. Rounds are processed in PAIRS packed into partition
  halves: even round -> h3 rows 0-63, odd round -> rows 64-127, so ACT/DVE
  per-element work is halved.
- Odd rounds are padded to the even round's width with DUPLICATE edges of
  the same column: harmless for max/min, excluded from the sum by exact
  matmul widths (bottom-half sum of deg-1 columns is polluted but the host
  ignores it).
- Destination nodes are assigned to cores round-robin in descending degree
  order, so all cores' degree histograms nearly coincide and the shared
  schedule pads only ~2% (incl. odd-round dup padding).
- Segment SUM via identity-matmul accumulation of h3 into a persistent
  PSUM bank; MAX and MIN as running bf16 tensor_tensor ops on DVE; h1
  relu on DVE (tensor_scalar add+max from PSUM), h2 relu and the h3
  PSUM->SBUF move on ACT — this balances DVE/ACT, the two busiest engines.
- Device emits raw per-column accumulators (sum/max/min, bf16); host
  finalizes (top+bottom combine, /deg, +b3, unpermute, concat with x and
  u[batch] passthrough columns).
"""

import numpy as np

import concourse.bass as bass

F32 = mybir.dt.float32
BF16 = mybir.dt.bfloat16
AF = mybir.ActivationFunctionType
ALU = mybir.AluOpType

P = 128
TW = 512  # tile width (columns = destination nodes)

N_NODES = 50000
N_EDGES = 800000
IN_CH = 64
HID_CH = 128
LAT_CH = 64
N_GRAPHS = 64
U_DIM = 32
N_CORES = 8


def build_schedule(row, col, n_nodes, n_cores):
    """Shared (SPMD) schedule + per-core column->node maps.

    Destination nodes are assigned to cores round-robin in descending
    degree order, so every core's degree histogram is within 1 of every
    other's and the shared max-histogram schedule has ~0.3% padding
    (vs ~8% for contiguous node ranges). Ownership is free to choose:
    the host assembles the output by global node id.
    """
    ncn = n_nodes // n_cores
    deg_all = np.bincount(col, minlength=n_nodes).astype(np.int64)
    dmax = int(deg_all.max())

    by_deg = np.argsort(-deg_all, kind="stable")  # all nodes, degree desc
    own_nodes = [by_deg[c::n_cores] for c in range(n_cores)]

    hist = np.zeros((n_cores, dmax + 1), np.int64)
    for c in range(n_cores):
        hist[c] = np.bincount(deg_all[own_nodes[c]], minlength=dmax + 1)
    H = hist.max(axis=0)

    # shared column degree sequence, descending
    col_degs = np.repeat(np.arange(dmax, 0, -1), H[dmax:0:-1])
    n_cols = len(col_degs)
    n_tiles = (n_cols + TW - 1) // TW
    ncp = n_tiles * TW

    # CSR of edges by destination (stable order)
    order = np.argsort(col, kind="stable")
    starts = np.zeros(n_nodes + 1, np.int64)
    np.cumsum(deg_all, out=starts[1:])
    src_of = row[order]  # src_of[starts[n] + r] = source of r-th edge into n

    # per-degree column block offsets in the shared sequence
    off = np.zeros(dmax + 2, np.int64)
    acc = 0
    for d in range(dmax, 0, -1):
        off[d] = acc
        acc += H[d]

    # per-core: column j -> global node id (or -1 virtual)
    col_node = np.full((n_cores, n_cols), -1, np.int64)
    for c in range(n_cores):
        own = own_nodes[c]  # already degree-descending
        d_own = deg_all[own]
        for d in range(1, dmax + 1):
            nodes_d = own[d_own == d]
            if len(nodes_d):
                col_node[c, off[d] : off[d] + len(nodes_d)] = nodes_d

    # tiles: (j0, j1, [(we, wo), ...]) + per-tile stream sizes
    tiles = []
    S = 0
    for t in range(n_tiles):
        j0, j1 = t * TW, min((t + 1) * TW, n_cols)
        degs = col_degs[j0:j1]
        d_t = int(degs[0])
        widths = [int(np.searchsorted(-degs, -(r + 1), side="right")) for r in range(d_t)]
        pairs = []
        s_t = 0
        for p in range((d_t + 1) // 2):
            we = widths[2 * p]
            wo = widths[2 * p + 1] if 2 * p + 1 < d_t else 0
            pairs.append((we, wo))
            s_t += 2 * we if wo > 0 else we
        tiles.append((j0, j1, pairs, S, s_t))
        S += s_t

    # per-tile max scheduled degree (for host finalize: bottom-half validity)
    tile_dt = np.array([len(prs) * 2 - (1 if prs[-1][1] == 0 else 0)
                        for _, _, prs, _, _ in tiles], np.int64)

    return dict(
        ncn=ncn, deg_all=deg_all, dmax=dmax, col_degs=col_degs, n_cols=n_cols,
        n_tiles=n_tiles, ncp=ncp, starts=starts, src_of=src_of,
        col_node=col_node, tiles=tiles, S=S, tile_dt=tile_dt,
    )


def build_stream_indices(sched, c, n_nodes):
    """Per-core gather indices for the [128, S] stream (ZERO = n_nodes)."""
    ZERO = n_nodes
    S = sched["S"]
    deg_all = sched["deg_all"]
    starts = sched["starts"]
    src_of = sched["src_of"]
    top = np.empty(S, np.int64)
    bot = np.empty(S, np.int64)
    o = 0
    for (j0, j1, pairs, _, _) in sched["tiles"]:
        nodes = sched["col_node"][c, j0:j1]
        real = nodes >= 0
        nn_ = np.where(real, nodes, 0)
        d = np.maximum(np.where(real, deg_all[nn_], 1), 1)
        st = starts[nn_]
        bcol = np.where(real, nn_, ZERO)
        for p, (we, wo) in enumerate(pairs):
            for r, wblk in ((2 * p, we), (2 * p + 1, we if wo > 0 else 0)):
                if wblk == 0:
                    continue
                rr = np.where(r < d[:wblk], r, r % d[:wblk])
                src = np.where(real[:wblk], src_of[st[:wblk] + rr], ZERO)
                top[o : o + wblk] = src
                bot[o : o + wblk] = bcol[:wblk]
                o += wblk
    assert o == S
    return top, bot


def build_kernel(sched, mode="full", reps=1, lookahead=0, psb=None):
    """Emit the Bass program (shared across cores).

    mode: 'full' | 'dmaonly' (stream DMAs only) | 'nomaxmin' (skip max/min
    DVE ops) | 'noisum' (skip sum matmuls) | 'norelu1' (h1 relu on ACT
    instead of DVE). reps>1 repeats the whole tile loop (slope timing).
    lookahead=1 software-pipelines: pair k+1's W1 matmuls are emitted
    before pair k's W2/W3/reduction tail so the PE never idles waiting on
    the DVE/ACT round trips inside a pair.
    """
    if psb is None:
        psb = (4, 2, 1, 1) if lookahead else (2, 2, 2, 2)
    nc = bass.Bass()

    S = sched["S"]
    ncp = sched["ncp"]
    tiles = sched["tiles"]
    s_t_max = max(s_t for _, _, _, _, s_t in tiles)

    xs_t = nc.dram_tensor("xstream", [P, S], BF16, kind="ExternalInput")
    w1_t = nc.dram_tensor("w1", [P, HID_CH], BF16, kind="ExternalInput")
    w2_t = nc.dram_tensor("w2", [HID_CH, HID_CH], BF16, kind="ExternalInput")
    w3_t = nc.dram_tensor("w3", [HID_CH, LAT_CH], BF16, kind="ExternalInput")
    i128_t = nc.dram_tensor("i128", [P, P], BF16, kind="ExternalInput")
    i64_t = nc.dram_tensor("i64", [64, 64], BF16, kind="ExternalInput")
    b1_t = nc.dram_tensor("b1", [HID_CH, 1], F32, kind="ExternalInput")
    b2_t = nc.dram_tensor("b2", [HID_CH, 1], F32, kind="ExternalInput")

    osum_t = nc.dram_tensor("out_sum", [P, ncp], BF16, kind="ExternalOutput")
    omax_t = nc.dram_tensor("out_max", [P, ncp], BF16, kind="ExternalOutput")
    omin_t = nc.dram_tensor("out_min", [P, ncp], BF16, kind="ExternalOutput")

    # flat pair list: (seq-tile idx, first, last, we, wo, stream offset)
    tiles_reps = list(tiles) * reps
    flat = []
    for ti, (j0, j1, pairs, s_off, s_t) in enumerate(tiles_reps):
        o = 0
        for p, (we, wo) in enumerate(pairs):
            flat.append((ti, p == 0, p == len(pairs) - 1, p, we, wo, o))
            o += 2 * we if wo > 0 else we

    with tile.TileContext(nc) as tc:
        with (
            tc.tile_pool(name="const", bufs=1) as constp,
            tc.tile_pool(name="stream", bufs=2) as streamp,
            tc.tile_pool(name="h1s", bufs=3) as h1sp,
            tc.tile_pool(name="h2s", bufs=3) as h2sp,
            tc.tile_pool(name="h3s", bufs=3) as h3sp,
            tc.tile_pool(name="vmx", bufs=2) as vmaxp,
            tc.tile_pool(name="vmn", bufs=2) as vminp,
            tc.tile_pool(name="sums", bufs=2) as sumsp,
            tc.tile_pool(name="ps_h1", bufs=psb[0], space="PSUM") as ps_h1,
            tc.tile_pool(name="ps_h2", bufs=psb[1], space="PSUM") as ps_h2,
            tc.tile_pool(name="ps_h3", bufs=psb[2], space="PSUM") as ps_h3,
            tc.tile_pool(name="ps_sum", bufs=psb[3], space="PSUM") as ps_sum,
        ):
            w1 = constp.tile([P, HID_CH], BF16); nc.sync.dma_start(w1[:], w1_t[:])
            w2 = constp.tile([HID_CH, HID_CH], BF16); nc.sync.dma_start(w2[:], w2_t[:])
            w3 = constp.tile([HID_CH, LAT_CH], BF16); nc.sync.dma_start(w3[:], w3_t[:])
            i128 = constp.tile([P, P], BF16); nc.sync.dma_start(i128[:], i128_t[:])
            i64 = constp.tile([64, 64], BF16); nc.sync.dma_start(i64[:], i64_t[:])
            b1 = constp.tile([HID_CH, 1], F32); nc.sync.dma_start(b1[:], b1_t[:])
            b2 = constp.tile([HID_CH, 1], F32); nc.sync.dma_start(b2[:], b2_t[:])

            tstate = {}

            def stageA(item):
                ti, first, _last, _p, we, wo, o = item
                if first:
                    j0, j1, pairs, s_off, s_t = tiles_reps[ti]
                    xs = streamp.tile([P, s_t_max], BF16, tag="xs")
                    nc.sync.dma_start(xs[:, :s_t], xs_t[:, s_off : s_off + s_t])
                    tstate[ti] = dict(xs=xs, j0=j0, we0=pairs[0][0])
                xs = tstate[ti]["xs"]
                h1pe = ps_h1.tile([P, TW], F32, tag="h1p")
                nc.tensor.matmul(h1pe[:, :we], lhsT=w1[:], rhs=xs[:, o : o + we],
                                 start=True, stop=True)
                h1po = None
                if wo > 0:
                    h1po = ps_h1.tile([P, TW], F32, tag="h1p")
                    nc.tensor.matmul(h1po[:, :we], lhsT=w1[:], rhs=xs[:, o + we : o + 2 * we],
                                     start=True, stop=True)
                return h1pe, h1po

            def relu1(dst, src, we):
                if mode == "norelu1":
                    nc.scalar.activation(dst[:, :we], src[:, :we], AF.Relu, bias=b1[:])
                else:
                    nc.vector.tensor_scalar(dst[:, :we], src[:, :we],
                                            b1[:, 0:1], 0.0, ALU.add, ALU.max)

            def stageB(item, h1pe, h1po):
                ti, first, last, p, we, wo, o = item
                st = tstate[ti]
                if first:
                    if mode != "noisum":
                        sumP = ps_sum.tile([P, TW], F32, tag="sumP")
                        st["sumP"] = sumP
                    if mode != "nomaxmin":
                        vmax = vmaxp.tile([P, TW], BF16, tag="vmax")
                        vmin = vminp.tile([P, TW], BF16, tag="vmin")
                        st["vmax"] = vmax
                        st["vmin"] = vmin
                sumP = st.get("sumP")
                vmax = st.get("vmax")
                vmin = st.get("vmin")

                h1e = h1sp.tile([P, TW], BF16, tag="h1")
                relu1(h1e, h1pe, we)
                h2pe = ps_h2.tile([P, TW], F32, tag="h2p")
                nc.tensor.matmul(h2pe[:, :we], lhsT=w2[:], rhs=h1e[:, :we], start=True, stop=True)
                h2e = h2sp.tile([P, TW], BF16, tag="h2")
                nc.scalar.activation(h2e[:, :we], h2pe[:, :we], AF.Relu, bias=b2[:])

                if wo > 0:
                    h1o = h1sp.tile([P, TW], BF16, tag="h1")
                    relu1(h1o, h1po, we)
                    h2po = ps_h2.tile([P, TW], F32, tag="h2p")
                    nc.tensor.matmul(h2po[:, :we], lhsT=w2[:], rhs=h1o[:, :we], start=True, stop=True)
                    h2o = h2sp.tile([P, TW], BF16, tag="h2")
                    nc.scalar.activation(h2o[:, :we], h2po[:, :we], AF.Relu, bias=b2[:])

                # h3: pack even -> rows 0-63, odd -> rows 64-127
                h3p = ps_h3.tile([P, TW], F32, tag="h3p")
                nc.tensor.matmul(h3p[0:64, :we], lhsT=w3[:], rhs=h2e[:, :we], start=True, stop=True)
                if wo > 0:
                    nc.tensor.matmul(h3p[64:P, :we], lhsT=w3[:], rhs=h2o[:, :we], start=True, stop=True)
                rows = P if wo > 0 else 64
                h3s = h3sp.tile([P, TW], BF16, tag="h3s")
                nc.scalar.activation(h3s[:rows, :we], h3p[:rows, :we], AF.Copy)

                # running max/min (DVE)
                if mode == "nomaxmin":
                    pass
                elif p == 0:
                    nc.vector.tensor_copy(vmax[:rows, :we], h3s[:rows, :we])
                    nc.vector.tensor_copy(vmin[:rows, :we], h3s[:rows, :we])
                else:
                    nc.vector.tensor_tensor(vmax[:rows, :we], vmax[:rows, :we],
                                            h3s[:rows, :we], ALU.max)
                    nc.vector.tensor_tensor(vmin[:rows, :we], vmin[:rows, :we],
                                            h3s[:rows, :we], ALU.min)

                # segment sum accumulation (identity matmuls)
                if mode == "noisum":
                    pass
                elif p == 0:
                    # full width incl. dup tail: bottom half of deg-1 columns
                    # is polluted; host ignores it.
                    nc.tensor.matmul(sumP[:rows, :we], lhsT=(i128 if rows == P else i64)[:],
                                     rhs=h3s[:rows, :we], start=True, stop=last,
                                     skip_group_check=True)
                else:
                    if wo > 0:
                        nc.tensor.matmul(sumP[:, :wo], lhsT=i128[:], rhs=h3s[:, :wo],
                                         start=False, stop=last, skip_group_check=True)
                        if we > wo:
                            nc.tensor.matmul(sumP[0:64, wo:we], lhsT=i64[:],
                                             rhs=h3s[0:64, wo:we],
                                             start=False, stop=last, skip_group_check=True)
                    else:
                        nc.tensor.matmul(sumP[0:64, :we], lhsT=i64[:], rhs=h3s[0:64, :we],
                                         start=False, stop=last, skip_group_check=True)

                if last:
                    j0, we0 = st["j0"], st["we0"]
                    if mode != "noisum":
                        sums = sumsp.tile([P, TW], BF16, tag="sums")
                        nc.scalar.activation(sums[:, :we0], sumP[:, :we0], AF.Copy)
                        nc.sync.dma_start(osum_t[:, j0 : j0 + we0], sums[:, :we0])
                    if mode != "nomaxmin":
                        nc.sync.dma_start(omax_t[:, j0 : j0 + we0], vmax[:, :we0])
                        nc.sync.dma_start(omin_t[:, j0 : j0 + we0], vmin[:, :we0])
                    del tstate[ti]

            if mode == "dmaonly":
                for ti, (j0, j1, pairs, s_off, s_t) in enumerate(tiles_reps):
                    xs = streamp.tile([P, s_t_max], BF16, tag="xs")
                    nc.sync.dma_start(xs[:, :s_t], xs_t[:, s_off : s_off + s_t])
                    nc.sync.dma_start(omax_t[:, j0 : j0 + 1], xs[0:P, 0:1])
            elif lookahead:
                prev = None
                for item in flat:
                    cur = stageA(item)
                    if prev is not None:
                        stageB(prev[0], prev[1], prev[2])
                    prev = (item, cur[0], cur[1])
                stageB(prev[0], prev[1], prev[2])
            else:
                for item in flat:
                    h1pe, h1po = stageA(item)
                    stageB(item, h1pe, h1po)
    return nc


# ---------------- public entry point ----------------


def kernel(**inputs):
    """Full-input NodeModel forward. Returns [N_NODES, 288] float32."""
    import ml_dtypes
    from concourse.bass_utils import run_bass_kernel_spmd

    BF = ml_dtypes.bfloat16

    x = np.asarray(inputs["x"], np.float32)
    edge_index = np.asarray(inputs["edge_index"])
    u = np.asarray(inputs["u"], np.float32)
    batch = np.asarray(inputs["batch"]).astype(np.int64)
    W1 = np.asarray(inputs["W1"], np.float32)
    b1 = np.asarray(inputs["b1"], np.float32)
    W2 = np.asarray(inputs["W2"], np.float32)
    b2 = np.asarray(inputs["b2"], np.float32)
    W3 = np.asarray(inputs["W3"], np.float32)
    b3 = np.asarray(inputs["b3"], np.float32)

    n_nodes = x.shape[0]
    row = edge_index[0].astype(np.int64)
    col = edge_index[1].astype(np.int64)

    sched = build_schedule(row, col, n_nodes, N_CORES)
    nc = build_kernel(sched)

    xT_aug = np.concatenate(
        [np.ascontiguousarray(x.T).astype(BF), np.zeros((IN_CH, 1), BF)], axis=1
    )

    in_maps = []
    for c in range(N_CORES):
        top, bot = build_stream_indices(sched, c, n_nodes)
        xs = np.empty((P, sched["S"]), BF)
        xs[0:64] = xT_aug[:, top]
        xs[64:128] = xT_aug[:, bot]
        in_maps.append({
            "xstream": xs,
            "w1": W1.astype(BF), "w2": W2.astype(BF), "w3": W3.astype(BF),
            "i128": np.eye(P, dtype=BF), "i64": np.eye(64, dtype=BF),
            "b1": np.ascontiguousarray(b1[:, None]),
            "b2": np.ascontiguousarray(b2[:, None]),
        })

    res = run_bass_kernel_spmd(nc, in_maps, core_ids=list(range(N_CORES)))

    # ---- host finalize ----
    out = np.zeros((n_nodes, 288), np.float32)
    out[:, 0:64] = x
    out[:, 256:288] = u[batch]

    deg_all = sched["deg_all"]
    tile_dt = sched["tile_dt"]
    for c in range(N_CORES):
        r = res.results[c]
        osum = np.asarray(r["out_sum"], np.float32)
        omax = np.asarray(r["out_max"]).astype(np.float32)
        omin = np.asarray(r["out_min"]).astype(np.float32)
        cn = sched["col_node"][c]
        j = np.where(cn >= 0)[0]
        nodes = cn[j]
        dj = deg_all[nodes].astype(np.float32)  # >= 1
        bv_mm = tile_dt[j // TW] >= 2          # max/min bottom valid (tile had odd rounds)
        bv_s = deg_all[nodes] >= 2             # sum bottom valid (col had odd rounds)

        s_top = osum[0:64, j]
        s_bot = np.where(bv_s[None, :], osum[64:128, j], 0.0)
        mean = ((s_top + s_bot) / dj[None, :]).T + b3[None, :]
        mx = np.maximum(omax[0:64, j],
                        np.where(bv_mm[None, :], omax[64:128, j], -np.inf)).T + b3[None, :]
        mn = np.minimum(omin[0:64, j],
                        np.where(bv_mm[None, :], omin[64:128, j], np.inf)).T + b3[None, :]
        out[nodes, 64:128] = mean
        out[nodes, 128:192] = mx
        out[nodes, 192:256] = mn
    return out


# revision 23
# speedup vs baseline: 11.5272x; 11.5272x over previous
"""Patch TileContext._drain_and_barrier: this container's walrus codegen
rejects >2 sem waits on one CTRL (Drain) instruction. Split the kernel-tail
drain's waits across separate nop instructions (1 wait each)."""
import concourse.tile as tile  # noqa
import concourse.mybir as mybir
from concourse.vector_clock import ScopedClock
from concourse._compat import not_none as nn


def _drain_and_barrier_split(self, tick_clock, wait_clock):
    nc = self.nc
    carrier = nc.sync.nop()
    wait_clock.add_sem_waits(carrier.ins, ScopedClock({None: tick_clock.global_clock}))
    si = carrier.ins.sync_info
    waits = list(si.on_wait) if si and si.on_wait else []
    if len(waits) > 1:
        si.on_wait.clear()
        si.on_wait.append(waits[0])
        for w in waits[1:]:
            n2 = nc.sync.nop()
            n2.ins.sync_info = mybir.SyncInfo(on_wait=[w], on_update=[])
    nc.sync.drain()

    nc.all_engine_barrier()
    assert self.sems is not None
    popped = nc._tile_sem_poison_stack.pop()
    assert popped is self._sem_poison
    nc.clear_and_free_semaphores(list(self.sems.allocated().values()))
    nc.all_engine_barrier()


tile.TileContext._drain_and_barrier = _drain_and_barrier_split


# ---- global wait-splitting pass ----
# This walrus build packs at most MAX_WAITS sem-waits per instruction
# (ISA EVENTS struct holds one; codegen can prepend a limited number of
# sync-wait commands). Move excess waits onto InstNoOp carriers.
MAX_WAITS = 2

def fix_waits(nc, max_waits=MAX_WAITS):
    import concourse.mybir as mybir
    dma2 = getattr(nc, "_fix_dma_waits2", False)
    n_fixed = 0
    for fn in nc.m.functions:
        for blk in fn.blocks:
            insts = blk.instructions
            out = []
            for inst in insts:
                lim = max_waits
                if dma2 and isinstance(inst, mybir.InstDMACopy):
                    lim = 2
                si = getattr(inst, "sync_info", None)
                if si is not None and si.on_wait and len(si.on_wait) > lim:
                    waits = list(si.on_wait)
                    si.on_wait.clear()
                    for w in waits[:-lim] if lim else waits:
                        n_fixed += 1
                        nop = mybir.InstNoOp(
                            name=f"{inst.name}.wsplit{n_fixed}",
                            sync_info=mybir.SyncInfo(on_wait=[w], on_update=[]),
                            bass_nofuse=True,
                            engine=inst.engine,
                        )
                        out.append(nop)
                    for w in waits[-lim:] if lim else []:
                        si.on_wait.append(w)
                out.append(inst)
            blk.instructions = out
    return n_fixed


# auto-apply fix_waits on serialization
import concourse.bass as _bass
_orig_to_json_bytes = _bass.Bass.to_json_bytes

def _to_json_bytes_fixed(self, *a, **kw):
    try:
        fix_waits(self, max_waits=getattr(self, "_fix_max_waits", 1))
    except Exception:
        import traceback; traceback.print_exc()
    return _orig_to_json_bytes(self, *a, **kw)

_bass.Bass.to_json_bytes = _to_json_bytes_fixed


"""NodeModel GNN kernel for Trainium2 (Bass/Tile), 8-core SPMD — v2.

Design (vs v1 baseline):
- Destination nodes sharded 8 ways (contiguous ranges, no collectives).
- Host pre-gathers & pre-transposes the per-edge source features into a
  bf16 stream [128, S]: rows 0-63 = x[src]^T per slot, rows 64-127 =
  x[dst]^T per slot. Sequential DMA at full bandwidth replaces the
  descriptor-rate-limited indirect gathers of v1.
- All matmuls bf16 (4x PE throughput vs fp32): h1 = relu(W1^T @ [xs;xc])
  is ONE K=128 matmul per round since lhsT = W1 as-is.
- Columns = destination nodes grouped by exact degree (shared histogram-max
  schedule across cores for SPMD); rounds process the r-th edge of each
  still-active column. Rounds are processed in PAIRS packed into partition
  halves: even round -> h3 rows 0-63, odd round -> rows 64-127, so ACT/DVE
  per-element work is halved.
- Odd rounds are padded to the even round's width with DUPLICATE edges of
  the same column: harmless for max/min, excluded from the sum by exact
  matmul widths (bottom-half sum of deg-1 columns is polluted but the host
  ignores it).
- Destination nodes are assigned to cores round-robin in descending degree
  order, so all cores' degree histograms nearly coincide and the shared
  schedule pads only ~2% (incl. odd-round dup padding).
- Segment SUM via identity-matmul accumulation of h3 into a persistent
  PSUM bank; MAX and MIN as running bf16 tensor_tensor ops on DVE; h1
  relu on DVE (tensor_scalar add+max from PSUM), h2 relu and the h3
  PSUM->SBUF move on ACT — this balances DVE/ACT, the two busiest engines.
- Device emits raw per-column accumulators (sum/max/min, bf16); host
  finalizes (top+bottom combine, /deg, +b3, unpermute, concat with x and
  u[batch] passthrough columns).
"""

import numpy as np

import concourse.bass as bass

F32 = mybir.dt.float32
BF16 = mybir.dt.bfloat16
AF = mybir.ActivationFunctionType
ALU = mybir.AluOpType

P = 128
TW = 512  # tile width (columns = destination nodes)
OWNERSHIP = "rr"  # "rr": round-robin by degree (tight schedule); "contig": v1-style

N_NODES = 50000
N_EDGES = 800000
IN_CH = 64
HID_CH = 128
LAT_CH = 64
N_GRAPHS = 64
U_DIM = 32
N_CORES = 8


def build_schedule(row, col, n_nodes, n_cores):
    """Shared (SPMD) schedule + per-core column->node maps.

    Destination nodes are assigned to cores round-robin in descending
    degree order, so every core's degree histogram is within 1 of every
    other's and the shared max-histogram schedule has ~0.3% padding
    (vs ~8% for contiguous node ranges). Ownership is free to choose:
    the host assembles the output by global node id.
    """
    ncn = n_nodes // n_cores
    deg_all = np.bincount(col, minlength=n_nodes).astype(np.int64)
    dmax = int(deg_all.max())

    if OWNERSHIP == "rr":
        by_deg = np.argsort(-deg_all, kind="stable")  # all nodes, degree desc
        own_nodes = [by_deg[c::n_cores] for c in range(n_cores)]
    else:  # contiguous ranges (larger histogram-max padding)
        own_nodes = []
        for c in range(n_cores):
            rng = np.arange(c * ncn, (c + 1) * ncn)
            own_nodes.append(rng[np.argsort(-deg_all[rng], kind="stable")])

    hist = np.zeros((n_cores, dmax + 1), np.int64)
    for c in range(n_cores):
        hist[c] = np.bincount(deg_all[own_nodes[c]], minlength=dmax + 1)
    H = hist.max(axis=0)

    # shared column degree sequence, descending
    col_degs = np.repeat(np.arange(dmax, 0, -1), H[dmax:0:-1])
    n_cols = len(col_degs)
    n_tiles = (n_cols + TW - 1) // TW
    ncp = n_tiles * TW

    # CSR of edges by destination (stable order)
    order = np.argsort(col, kind="stable")
    starts = np.zeros(n_nodes + 1, np.int64)
    np.cumsum(deg_all, out=starts[1:])
    src_of = row[order]  # src_of[starts[n] + r] = source of r-th edge into n

    # per-degree column block offsets in the shared sequence
    off = np.zeros(dmax + 2, np.int64)
    acc = 0
    for d in range(dmax, 0, -1):
        off[d] = acc
        acc += H[d]

    # per-core: column j -> global node id (or -1 virtual)
    col_node = np.full((n_cores, n_cols), -1, np.int64)
    for c in range(n_cores):
        own = own_nodes[c]  # already degree-descending
        d_own = deg_all[own]
        for d in range(1, dmax + 1):
            nodes_d = own[d_own == d]
            if len(nodes_d):
                col_node[c, off[d] : off[d] + len(nodes_d)] = nodes_d

    # tiles: (j0, j1, [(we, wo), ...]) + per-tile stream sizes
    tiles = []
    S = 0
    for t in range(n_tiles):
        j0, j1 = t * TW, min((t + 1) * TW, n_cols)
        degs = col_degs[j0:j1]
        d_t = int(degs[0])
        widths = [int(np.searchsorted(-degs, -(r + 1), side="right")) for r in range(d_t)]
        pairs = []
        s_t = 0
        for p in range((d_t + 1) // 2):
            we = widths[2 * p]
            wo = widths[2 * p + 1] if 2 * p + 1 < d_t else 0
            pairs.append((we, wo))
            s_t += 2 * we if wo > 0 else we
        tiles.append((j0, j1, pairs, S, s_t))
        S += s_t

    # per-tile max scheduled degree (for host finalize: bottom-half validity)
    tile_dt = np.array([len(prs) * 2 - (1 if prs[-1][1] == 0 else 0)
                        for _, _, prs, _, _ in tiles], np.int64)

    return dict(
        ncn=ncn, deg_all=deg_all, dmax=dmax, col_degs=col_degs, n_cols=n_cols,
        n_tiles=n_tiles, ncp=ncp, starts=starts, src_of=src_of,
        col_node=col_node, tiles=tiles, S=S, tile_dt=tile_dt,
    )


def build_stream_indices(sched, c, n_nodes):
    """Per-core gather indices for the [128, S] stream (ZERO = n_nodes)."""
    ZERO = n_nodes
    S = sched["S"]
    deg_all = sched["deg_all"]
    starts = sched["starts"]
    src_of = sched["src_of"]
    top = np.empty(S, np.int64)
    bot = np.empty(S, np.int64)
    o = 0
    for (j0, j1, pairs, _, _) in sched["tiles"]:
        nodes = sched["col_node"][c, j0:j1]
        real = nodes >= 0
        nn_ = np.where(real, nodes, 0)
        d = np.maximum(np.where(real, deg_all[nn_], 1), 1)
        st = starts[nn_]
        bcol = np.where(real, nn_, ZERO)
        for p, (we, wo) in enumerate(pairs):
            for r, wblk in ((2 * p, we), (2 * p + 1, we if wo > 0 else 0)):
                if wblk == 0:
                    continue
                rr = np.where(r < d[:wblk], r, r % d[:wblk])
                src = np.where(real[:wblk], src_of[st[:wblk] + rr], ZERO)
                top[o : o + wblk] = src
                bot[o : o + wblk] = bcol[:wblk]
                o += wblk
    assert o == S
    return top, bot


def build_kernel(sched, mode="full", reps=1, lookahead=0, psb=None):
    """Emit the Bass program (shared across cores).

    mode: 'full' | 'dmaonly' (stream DMAs only) | 'nomaxmin' (skip max/min
    DVE ops) | 'noisum' (skip sum matmuls) | 'norelu1' (h1 relu on ACT
    instead of DVE). reps>1 repeats the whole tile loop (slope timing).
    lookahead=1 software-pipelines: pair k+1's W1 matmuls are emitted
    before pair k's W2/W3/reduction tail so the PE never idles waiting on
    the DVE/ACT round trips inside a pair.
    """
    if psb is None:
        psb = (4, 2, 1, 1) if lookahead else (2, 2, 2, 2)
    nc = bass.Bass()

    S = sched["S"]
    ncp = sched["ncp"]
    tiles = sched["tiles"]
    s_t_max = max(s_t for _, _, _, _, s_t in tiles)

    xs_t = nc.dram_tensor("xstream", [P, S], BF16, kind="ExternalInput")
    w1_t = nc.dram_tensor("w1", [P, HID_CH], BF16, kind="ExternalInput")
    w2_t = nc.dram_tensor("w2", [HID_CH, HID_CH], BF16, kind="ExternalInput")
    w3_t = nc.dram_tensor("w3", [HID_CH, LAT_CH], BF16, kind="ExternalInput")
    i128_t = nc.dram_tensor("i128", [P, P], BF16, kind="ExternalInput")
    i64_t = nc.dram_tensor("i64", [64, 64], BF16, kind="ExternalInput")
    b1_t = nc.dram_tensor("b1", [HID_CH, 1], F32, kind="ExternalInput")
    b2_t = nc.dram_tensor("b2", [HID_CH, 1], F32, kind="ExternalInput")

    osum_t = nc.dram_tensor("out_sum", [P, ncp], BF16, kind="ExternalOutput")
    omax_t = nc.dram_tensor("out_max", [P, ncp], BF16, kind="ExternalOutput")
    omin_t = nc.dram_tensor("out_min", [P, ncp], BF16, kind="ExternalOutput")

    # flat pair list: (seq-tile idx, first, last, we, wo, stream offset)
    tiles_reps = list(tiles) * reps
    flat = []
    for ti, (j0, j1, pairs, s_off, s_t) in enumerate(tiles_reps):
        o = 0
        for p, (we, wo) in enumerate(pairs):
            flat.append((ti, p == 0, p == len(pairs) - 1, p, we, wo, o))
            o += 2 * we if wo > 0 else we

    with tile.TileContext(nc) as tc:
        with (
            tc.tile_pool(name="const", bufs=1) as constp,
            tc.tile_pool(name="stream", bufs=2) as streamp,
            tc.tile_pool(name="h1s", bufs=3) as h1sp,
            tc.tile_pool(name="h2s", bufs=3) as h2sp,
            tc.tile_pool(name="h3s", bufs=3) as h3sp,
            tc.tile_pool(name="vmx", bufs=2) as vmaxp,
            tc.tile_pool(name="vmn", bufs=2) as vminp,
            tc.tile_pool(name="sums", bufs=2) as sumsp,
            tc.tile_pool(name="ps_h1", bufs=psb[0], space="PSUM") as ps_h1,
            tc.tile_pool(name="ps_h2", bufs=psb[1], space="PSUM") as ps_h2,
            tc.tile_pool(name="ps_h3", bufs=psb[2], space="PSUM") as ps_h3,
            tc.tile_pool(name="ps_sum", bufs=psb[3], space="PSUM") as ps_sum,
        ):
            w1 = constp.tile([P, HID_CH], BF16); nc.sync.dma_start(w1[:], w1_t[:])
            w2 = constp.tile([HID_CH, HID_CH], BF16); nc.sync.dma_start(w2[:], w2_t[:])
            w3 = constp.tile([HID_CH, LAT_CH], BF16); nc.sync.dma_start(w3[:], w3_t[:])
            i128 = constp.tile([P, P], BF16); nc.sync.dma_start(i128[:], i128_t[:])
            i64 = constp.tile([64, 64], BF16); nc.sync.dma_start(i64[:], i64_t[:])
            b1 = constp.tile([HID_CH, 1], F32); nc.sync.dma_start(b1[:], b1_t[:])
            b2 = constp.tile([HID_CH, 1], F32); nc.sync.dma_start(b2[:], b2_t[:])

            tstate = {}

            def stageA(item):
                ti, first, _last, _p, we, wo, o = item
                if first:
                    j0, j1, pairs, s_off, s_t = tiles_reps[ti]
                    xs = streamp.tile([P, s_t_max], BF16, tag="xs")
                    nc.sync.dma_start(xs[:, :s_t], xs_t[:, s_off : s_off + s_t])
                    tstate[ti] = dict(xs=xs, j0=j0, we0=pairs[0][0])
                xs = tstate[ti]["xs"]
                h1pe = ps_h1.tile([P, TW], F32, tag="h1p")
                nc.tensor.matmul(h1pe[:, :we], lhsT=w1[:], rhs=xs[:, o : o + we],
                                 start=True, stop=True)
                h1po = None
                if wo > 0:
                    h1po = ps_h1.tile([P, TW], F32, tag="h1p")
                    nc.tensor.matmul(h1po[:, :we], lhsT=w1[:], rhs=xs[:, o + we : o + 2 * we],
                                     start=True, stop=True)
                return h1pe, h1po

            def relu1(dst, src, we):
                if mode == "norelu1":
                    nc.scalar.activation(dst[:, :we], src[:, :we], AF.Relu, bias=b1[:])
                else:
                    nc.vector.tensor_scalar(dst[:, :we], src[:, :we],
                                            b1[:, 0:1], 0.0, ALU.add, ALU.max)

            def stageB(item, h1pe, h1po):
                ti, first, last, p, we, wo, o = item
                st = tstate[ti]
                if first:
                    if mode != "noisum":
                        sumP = ps_sum.tile([P, TW], F32, tag="sumP")
                        st["sumP"] = sumP
                    if mode != "nomaxmin":
                        vmax = vmaxp.tile([P, TW], BF16, tag="vmax")
                        vmin = vminp.tile([P, TW], BF16, tag="vmin")
                        st["vmax"] = vmax
                        st["vmin"] = vmin
                sumP = st.get("sumP")
                vmax = st.get("vmax")
                vmin = st.get("vmin")

                h1e = h1sp.tile([P, TW], BF16, tag="h1")
                relu1(h1e, h1pe, we)
                h2pe = ps_h2.tile([P, TW], F32, tag="h2p")
                nc.tensor.matmul(h2pe[:, :we], lhsT=w2[:], rhs=h1e[:, :we], start=True, stop=True)
                h2e = h2sp.tile([P, TW], BF16, tag="h2")
                nc.scalar.activation(h2e[:, :we], h2pe[:, :we], AF.Relu, bias=b2[:])

                if wo > 0:
                    h1o = h1sp.tile([P, TW], BF16, tag="h1")
                    relu1(h1o, h1po, we)
                    h2po = ps_h2.tile([P, TW], F32, tag="h2p")
                    nc.tensor.matmul(h2po[:, :we], lhsT=w2[:], rhs=h1o[:, :we], start=True, stop=True)
                    h2o = h2sp.tile([P, TW], BF16, tag="h2")
                    nc.scalar.activation(h2o[:, :we], h2po[:, :we], AF.Relu, bias=b2[:])

                # h3: pack even -> rows 0-63, odd -> rows 64-127
                h3p = ps_h3.tile([P, TW], F32, tag="h3p")
                nc.tensor.matmul(h3p[0:64, :we], lhsT=w3[:], rhs=h2e[:, :we], start=True, stop=True)
                if wo > 0:
                    nc.tensor.matmul(h3p[64:P, :we], lhsT=w3[:], rhs=h2o[:, :we], start=True, stop=True)
                rows = P if wo > 0 else 64
                h3s = h3sp.tile([P, TW], BF16, tag="h3s")
                nc.scalar.activation(h3s[:rows, :we], h3p[:rows, :we], AF.Copy)

                # running max/min (DVE)
                if mode == "nomaxmin":
                    pass
                elif p == 0:
                    nc.vector.tensor_copy(vmax[:rows, :we], h3s[:rows, :we])
                    nc.vector.tensor_copy(vmin[:rows, :we], h3s[:rows, :we])
                else:
                    nc.vector.tensor_tensor(vmax[:rows, :we], vmax[:rows, :we],
                                            h3s[:rows, :we], ALU.max)
                    nc.vector.tensor_tensor(vmin[:rows, :we], vmin[:rows, :we],
                                            h3s[:rows, :we], ALU.min)

                # segment sum accumulation (identity matmuls)
                if mode == "noisum":
                    pass
                elif p == 0:
                    # full width incl. dup tail: bottom half of deg-1 columns
                    # is polluted; host ignores it.
                    nc.tensor.matmul(sumP[:rows, :we], lhsT=(i128 if rows == P else i64)[:],
                                     rhs=h3s[:rows, :we], start=True, stop=last,
                                     skip_group_check=True)
                else:
                    if wo > 0:
                        nc.tensor.matmul(sumP[:, :wo], lhsT=i128[:], rhs=h3s[:, :wo],
                                         start=False, stop=last, skip_group_check=True)
                        if we > wo:
                            nc.tensor.matmul(sumP[0:64, wo:we], lhsT=i64[:],
                                             rhs=h3s[0:64, wo:we],
                                             start=False, stop=last, skip_group_check=True)
                    else:
                        nc.tensor.matmul(sumP[0:64, :we], lhsT=i64[:], rhs=h3s[0:64, :we],
                                         start=False, stop=last, skip_group_check=True)

                if last:
                    j0, we0 = st["j0"], st["we0"]
                    if mode != "noisum":
                        sums = sumsp.tile([P, TW], BF16, tag="sums")
                        nc.scalar.activation(sums[:, :we0], sumP[:, :we0], AF.Copy)
                        nc.sync.dma_start(osum_t[:, j0 : j0 + we0], sums[:, :we0])
                    if mode != "nomaxmin":
                        nc.sync.dma_start(omax_t[:, j0 : j0 + we0], vmax[:, :we0])
                        nc.sync.dma_start(omin_t[:, j0 : j0 + we0], vmin[:, :we0])
                    del tstate[ti]

            if mode == "dmaonly":
                for ti, (j0, j1, pairs, s_off, s_t) in enumerate(tiles_reps):
                    xs = streamp.tile([P, s_t_max], BF16, tag="xs")
                    nc.sync.dma_start(xs[:, :s_t], xs_t[:, s_off : s_off + s_t])
                    nc.sync.dma_start(omax_t[:, j0 : j0 + 1], xs[0:P, 0:1])
            elif lookahead:
                prev = None
                for item in flat:
                    cur = stageA(item)
                    if prev is not None:
                        stageB(prev[0], prev[1], prev[2])
                    prev = (item, cur[0], cur[1])
                stageB(prev[0], prev[1], prev[2])
            else:
                for item in flat:
                    h1pe, h1po = stageA(item)
                    stageB(item, h1pe, h1po)
    return nc


# ---------------- public entry point ----------------


def kernel(**inputs):
    """Full-input NodeModel forward. Returns [N_NODES, 288] float32."""
    import ml_dtypes
    from concourse.bass_utils import run_bass_kernel_spmd

    BF = ml_dtypes.bfloat16

    x = np.asarray(inputs["x"], np.float32)
    edge_index = np.asarray(inputs["edge_index"])
    u = np.asarray(inputs["u"], np.float32)
    batch = np.asarray(inputs["batch"]).astype(np.int64)
    W1 = np.asarray(inputs["W1"], np.float32)
    b1 = np.asarray(inputs["b1"], np.float32)
    W2 = np.asarray(inputs["W2"], np.float32)
    b2 = np.asarray(inputs["b2"], np.float32)
    W3 = np.asarray(inputs["W3"], np.float32)
    b3 = np.asarray(inputs["b3"], np.float32)

    n_nodes = x.shape[0]
    row = edge_index[0].astype(np.int64)
    col = edge_index[1].astype(np.int64)

    sched = build_schedule(row, col, n_nodes, N_CORES)
    nc = build_kernel(sched)

    xT_aug = np.concatenate(
        [np.ascontiguousarray(x.T).astype(BF), np.zeros((IN_CH, 1), BF)], axis=1
    )

    in_maps = []
    for c in range(N_CORES):
        top, bot = build_stream_indices(sched, c, n_nodes)
        xs = np.empty((P, sched["S"]), BF)
        xs[0:64] = xT_aug[:, top]
        xs[64:128] = xT_aug[:, bot]
        in_maps.append({
            "xstream": xs,
            "w1": W1.astype(BF), "w2": W2.astype(BF), "w3": W3.astype(BF),
            "i128": np.eye(P, dtype=BF), "i64": np.eye(64, dtype=BF),
            "b1": np.ascontiguousarray(b1[:, None]),
            "b2": np.ascontiguousarray(b2[:, None]),
        })

    res = run_bass_kernel_spmd(nc, in_maps, core_ids=list(range(N_CORES)))

    # ---- host finalize ----
    out = np.zeros((n_nodes, 288), np.float32)
    out[:, 0:64] = x
    out[:, 256:288] = u[batch]

    deg_all = sched["deg_all"]
    tile_dt = sched["tile_dt"]
    for c in range(N_CORES):
        r = res.results[c]
        osum = np.asarray(r["out_sum"], np.float32)
        omax = np.asarray(r["out_max"]).astype(np.float32)
        omin = np.asarray(r["out_min"]).astype(np.float32)
        cn = sched["col_node"][c]
        j = np.where(cn >= 0)[0]
        nodes = cn[j]
        dj = deg_all[nodes].astype(np.float32)  # >= 1
        bv_mm = tile_dt[j // TW] >= 2          # max/min bottom valid (tile had odd rounds)
        bv_s = deg_all[nodes] >= 2             # sum bottom valid (col had odd rounds)

        s_top = osum[0:64, j]
        s_bot = np.where(bv_s[None, :], osum[64:128, j], 0.0)
        mean = ((s_top + s_bot) / dj[None, :]).T + b3[None, :]
        mx = np.maximum(omax[0:64, j],
                        np.where(bv_mm[None, :], omax[64:128, j], -np.inf)).T + b3[None, :]
        mn = np.minimum(omin[0:64, j],
                        np.where(bv_mm[None, :], omin[64:128, j], np.inf)).T + b3[None, :]
        out[nodes, 64:128] = mean
        out[nodes, 128:192] = mx
        out[nodes, 192:256] = mn
    return out
